# revision 1
# baseline (speedup 1.0000x reference)
"""Trainium2 Bass kernel for nn_AlignedQuesEmb (v2 — fp16 dataflow).

Reference computation (per batch element b):
    q_dense = relu(query @ W.T + bias)        [Q=48, 300]
    c_dense = relu(ctx @ W.T + bias)          [C=2048, 300]
    scores  = c_dense @ q_dense.T             [C, Q]
    align   = softmax(scores, axis=-1)        (over Q)
    out     = align @ query                   [C, 300]

Sharding: data-parallel over batch. B=64 -> 8 NeuronCores x 8 batches each.

v2 design (vs the f32r v1 baseline at ~239us):
  * All large HBM traffic is 16-bit. ctx is cast to fp16 on the host
    (10 explicit mantissa bits == f32r's PE-internal precision, and ctx
    values are O(1) so fp16 range is a non-issue); output is stored fp16
    (normalized on device) and upcast on the host. Halves both load and
    store traffic: ~20MB/pass/core vs ~41MB.
  * ctx for one batch is packed on the host as a single [100, 6144] fp16
    block (3 contraction bands of 100 partitions side by side), so each
    batch needs exactly ONE 1.23MB load DMA.
  * All big matmuls are 16-bit (fp16/bf16) with 1024-wide moving chunks.
    q_dense is computed once, exactly, in fp32 (its error otherwise
    dominates: scores reach |110| and softmax amplifies).
  * Softmax over q is the constant-shift trick from v1 (centered q_dense
    keeps scores in [-56, 110]; SHIFT=60 keeps exp and row sums finite in
    f32). E is bf16 (range!); ones-column in the padded query gives the
    row sums in mm3's column 300; a DVE reciprocal + ACT/DVE per-partition
    scale normalizes on-device.
  * PE program order is software-pipelined: mm1(b), mm3(b-1), mm2(b) --
    so PSUM drains and the exp for batch b-1 complete while mm1(b) runs,
    and the PE (the bottleneck engine) never waits on ACT/DVE.
  * ctx loads prefetch 2 batches ahead with a pool rotation (bufs=4,
    8 % 4 == 0) chosen so the schedule stays bubble-free across For_i
    iterations of the timing loop.
"""

import numpy as np
import ml_dtypes

try:
    import concourse.bass as bass  # noqa: F401
except ImportError:
    import sys
    sys.path.insert(0, "/opt/trn_rl_repo")

import concourse.bass as bass
import concourse.tile as tile
from concourse import bacc, mybir
from concourse import bass_utils

F32 = mybir.dt.float32
F16 = mybir.dt.float16
BF16 = mybir.dt.bfloat16
AF = mybir.ActivationFunctionType
AX = mybir.AxisListType
ALU = mybir.AluOpType

B, Q, C, D = 64, 48, 2048, 300
NCORES = 8
BPC = B // NCORES              # batches per core
SHIFT = 60.0                   # constant softmax shift (see module docstring)
KB = 100                       # contraction-band width (3 bands of 100 = D)
EBANDS = [(0, 128), (128, 128), (256, 44)]  # output-dim bands of c_dense
DP = 304                       # query padded with a ones column (row-sum fold)
NT = C // 128                  # c tiles of 128 for mm3


def _build(reps: int = 1, loop_reps: int = 1):
    nc = bacc.Bacc("TRN2", target_bir_lowering=False, debug=False)

    ctxp_d = nc.dram_tensor("ctxp", [BPC, KB, 3 * C], F16, kind="ExternalInput").ap()
    qtf_d = nc.dram_tensor("qtf", [KB, 3 * BPC * Q], F32, kind="ExternalInput").ap()
    wtf_d = nc.dram_tensor("wtf", [KB, 3 * D], F32, kind="ExternalInput").ap()
    wtq_d = nc.dram_tensor("wtq", [KB, 3 * D], F16, kind="ExternalInput").ap()
    qryb_d = nc.dram_tensor("qryb", [Q, BPC * DP], BF16, kind="ExternalInput").ap()
    bias_d = nc.dram_tensor("bias", [D, 1], F32, kind="ExternalInput").ap()
    out_d = nc.dram_tensor("out", [BPC, C, D], F16, kind="ExternalOutput").ap()

    with tile.TileContext(nc) as tc:
        with (
            tc.tile_pool(name="const", bufs=1) as const,
            tc.tile_pool(name="ctx", bufs=8) as ctxp,
            tc.tile_pool(name="cdT", bufs=2) as cdp,
            tc.tile_pool(name="esb", bufs=2) as esbp,
            tc.tile_pool(name="osb", bufs=2) as osbp,
            tc.tile_pool(name="rcp", bufs=2) as rcpp,
            tc.tile_pool(name="pcd", bufs=2, space="PSUM") as pcd,
            tc.tile_pool(name="psc", bufs=2, space="PSUM") as psc,
            tc.tile_pool(name="pout", bufs=2, space="PSUM") as pout,
        ):
            # ---- constants ----
            wtq = const.tile([KB, 3 * D], F16, tag="wtq")
            nc.sync.dma_start(wtq[:], wtq_d)
            wtf = const.tile([KB, 3 * D], F32, tag="wtf")
            nc.sync.dma_start(wtf[:], wtf_d)
            qtf = const.tile([KB, 3 * BPC * Q], F32, tag="qtf")
            nc.sync.dma_start(qtf[:], qtf_d)
            qryb = const.tile([Q, BPC * DP], BF16, tag="qryb")
            nc.sync.dma_start(qryb[:], qryb_d)
            bt = []
            for m, (e0, ep) in enumerate(EBANDS):
                btm = const.tile([ep, 1], F32, tag=f"bt{m}")
                nc.sync.dma_start(btm[:], bias_d[e0:e0 + ep, :])
                bt.append(btm)
            negshift = const.tile([Q, 1], F32, tag="negshift")
            nc.vector.memset(negshift[:], -SHIFT)

            # ---- q_denseT for all local batches (exact fp32), centered,
            #      rounded once to fp16 for mm2 ----
            qdT = []
            for m, (e0, ep) in enumerate(EBANDS):
                ps = pcd.tile([128, 1024], F32, tag="pcd", name=f"pq_{m}")
                for k in range(3):
                    nc.tensor.matmul(
                        ps[0:ep, 0:BPC * Q],
                        wtf[:, k * D + e0:k * D + e0 + ep],
                        qtf[:, k * BPC * Q:(k + 1) * BPC * Q],
                        start=(k == 0), stop=(k == 2),
                    )
                qf = const.tile([ep, BPC * Q], F32, tag=f"qdTf{m}")
                nc.scalar.activation(qf[:], ps[0:ep, 0:BPC * Q], AF.Relu,
                                     bias=bt[m][:])
                mean = const.tile([ep, 1], F32, tag=f"qmean{m}")
                nc.vector.reduce_sum(mean[:], qf[:], axis=AX.X)
                nc.vector.tensor_scalar_mul(mean[:], mean[:], 1.0 / (BPC * Q))
                nc.vector.tensor_scalar_sub(qf[:], qf[:], mean[:])
                q16 = const.tile([ep, BPC * Q], F16, tag=f"qdT{m}")
                nc.vector.tensor_copy(q16[:], qf[:])
                qdT.append(q16)

            def mm1(bb):
                """c_denseT [e, c] = relu(W.T-matmul(ctx) + b), fp16.

                The load is issued here but the sync queue runs it up to
                bufs=4 batches ahead of the consuming matmuls (the DMA only
                waits on its slot's WAR), so loads prefetch automatically,
                including across For_i iterations."""
                cx = ctxp.tile([KB, 3 * C], F16, tag="ctx", name=f"ctx_{bb}")
                nc.sync.dma_start(cx[:], ctxp_d[bb % BPC])
                cdT = [cdp.tile([ep, C], F16, tag=f"cd{m}", name=f"cd{m}_{bb}")
                       for m, (e0, ep) in enumerate(EBANDS)]
                for m, (e0, ep) in enumerate(EBANDS):
                    for g in range(2):
                        ps = pcd.tile([128, 1024], F32, tag="pcd",
                                      name=f"pcd_{bb}_{m}_{g}")
                        for k in range(3):
                            for j2 in range(2):
                                nc.tensor.matmul(
                                    ps[0:ep, j2 * 512:(j2 + 1) * 512],
                                    wtq[:, k * D + e0:k * D + e0 + ep],
                                    cx[:, k * C + g * 1024 + j2 * 512:
                                       k * C + g * 1024 + (j2 + 1) * 512],
                                    start=(k == 0), stop=(k == 2),
                                    skip_group_check=True,
                                )
                        dst = cdT[m][:, g * 1024:(g + 1) * 1024]
                        if g == 0:
                            nc.scalar.activation(dst, ps[0:ep, :], AF.Relu,
                                                 bias=bt[m][:])
                        else:
                            nc.vector.tensor_scalar(
                                dst, ps[0:ep, :], bt[m][:], 0.0,
                                ALU.add, ALU.max,
                            )
                return cdT

            # E tiles pre-allocated so the For_i body can read the previous
            # iteration's E[7] at block 0 (loop-carried software pipeline).
            E_tiles = [esbp.tile([Q, C], BF16, tag="E", name=f"E_{b}")
                       for b in range(BPC)]

            def mm2(bb, cdT):
                """scoresT [q, c] -> E = exp(scoresT - SHIFT), bf16."""
                E = E_tiles[bb % BPC]
                qsl = slice((bb % BPC) * Q, (bb % BPC + 1) * Q)
                for jj in range(4):
                    ps2 = psc.tile([Q, 512], F32, tag="psc",
                                   name=f"psc_{bb}_{jj}")
                    for m, (e0, ep) in enumerate(EBANDS):
                        nc.tensor.matmul(
                            ps2[:],
                            qdT[m][:, qsl],
                            cdT[m][:, jj * 512:(jj + 1) * 512],
                            start=(m == 0), stop=(m == 2),
                        )
                    nc.scalar.activation(
                        E[:, jj * 512:(jj + 1) * 512], ps2[:], AF.Exp,
                        bias=negshift[:],
                    )
                return E

            def mm3(bb, E):
                """out rows = (E.T @ query_pad) normalized by the ones-column
                row sum; fp16 store via one SWDGE DMA."""
                rc = rcpp.tile([128, NT], F32, tag="rc", name=f"rc_{bb}")
                osb = osbp.tile([128, NT * D], F16, tag="osb", name=f"osb_{bb}")
                qsl = slice((bb % BPC) * DP, (bb % BPC + 1) * DP)
                TG = 4   # tiles per store DMA: 4 stores/batch so completions
                # spread out (the staggered-reset stage preambles wait on
                # outstanding store DMAs; one big tail store stalls them)
                for t in range(NT):
                    po = pout.tile([128, DP], F32, tag="pout",
                                   name=f"pout_{bb}_{t}")
                    nc.tensor.matmul(
                        po[:], E[:, t * 128:(t + 1) * 128], qryb[:, qsl],
                        start=True, stop=True,
                    )
                    nc.vector.reciprocal(rc[:, t:t + 1], po[:, D:D + 1])
                    dst = osb[:, t * D:(t + 1) * D]
                    if t % 2 == 0:
                        nc.scalar.activation(dst, po[:, 0:D], AF.Copy,
                                             scale=rc[:, t:t + 1])
                    else:
                        nc.vector.tensor_scalar_mul(dst, po[:, 0:D],
                                                    rc[:, t:t + 1])
                    if t % TG == TG - 1:
                        g = t // TG
                        # stores go out on the SWDGE (POOL) queue so the SP
                        # HWDGE stream only carries loads.
                        nc.gpsimd.dma_start(
                            out_d[bb % BPC][g * TG * 128:(g + 1) * TG * 128, :]
                                .rearrange("(t p) d -> p t d", p=128),
                            osb[:, g * TG * D:(g + 1) * TG * D]
                                .rearrange("p (t d) -> p t d", t=TG),
                        )

            def one_pass(base):
                prev = None
                for gb in range(BPC):
                    bb = base + gb
                    cdT = mm1(bb)
                    if prev is not None:
                        mm3(*prev)
                    prev = (bb, mm2(bb, cdT))
                mm3(*prev)

            def one_pass_wrapped():
                """For_i body: batch 7's mm3 wraps to block 0 of the NEXT
                iteration, so the PE never runs an exposed mm3 tail (the
                first iteration emits one garbage out[7] store, overwritten
                by every later iteration)."""
                for gb in range(BPC):
                    cdT = mm1(gb)
                    mm3((gb - 1) % BPC, E_tiles[(gb - 1) % BPC])
                    mm2(gb, cdT)

            if loop_reps > 1:
                ET = mybir.EngineType
                with tc.For_i(0, loop_reps, 1, staggered_reset=True,
                              hint_engines=(ET.PE, ET.DVE, ET.Activation, ET.SP)):
                    one_pass_wrapped()
            else:
                for rep in range(reps):
                    one_pass(rep * BPC)
    nc.compile()
    return nc


def _prep_in_maps(query_emb, ctx_embed, W, b):
    query_emb = np.ascontiguousarray(query_emb, dtype=np.float32)
    ctx_embed = np.asarray(ctx_embed, dtype=np.float32)
    W = np.asarray(W, dtype=np.float32)
    wT = np.ascontiguousarray(W.T)                                # [d, e]
    wtf = np.ascontiguousarray(
        wT.reshape(3, KB, D).transpose(1, 0, 2).reshape(KB, 3 * D))
    wtq = wtf.astype(np.float16)
    bias = np.ascontiguousarray(np.asarray(b, np.float32).reshape(D, 1))
    in_maps = []
    for cix in range(NCORES):
        qc = query_emb[cix * BPC:(cix + 1) * BPC]                 # [BPC, Q, D]
        cc = ctx_embed[cix * BPC:(cix + 1) * BPC]                 # [BPC, C, D]
        ctxp = (cc.transpose(0, 2, 1)                             # [BPC, D, C]
                .reshape(BPC, 3, KB, C).transpose(0, 2, 1, 3)
                .reshape(BPC, KB, 3 * C).astype(np.float16))
        qT = qc.transpose(2, 0, 1).reshape(D, BPC * Q)            # [D, BPC*Q]
        qtf = np.ascontiguousarray(
            qT.reshape(3, KB, BPC * Q).transpose(1, 0, 2)
            .reshape(KB, 3 * BPC * Q))
        qp = np.zeros((BPC, Q, DP), np.float32)
        qp[:, :, :D] = qc
        qp[:, :, D] = 1.0     # ones column: mm3 also produces the row sums
        qryb = np.ascontiguousarray(
            qp.transpose(1, 0, 2).reshape(Q, BPC * DP)).astype(ml_dtypes.bfloat16)
        in_maps.append({
            "ctxp": np.ascontiguousarray(ctxp),
            "qtf": qtf,
            "wtf": wtf,
            "wtq": wtq,
            "qryb": qryb,
            "bias": bias,
        })
    return in_maps


_NC_CACHE = {}


def _get_nc(reps: int = 1):
    if reps not in _NC_CACHE:
        _NC_CACHE[reps] = _build(reps)
    return _NC_CACHE[reps]


def kernel(query_emb, ctx_embed, W, b):
    nc = _get_nc()
    in_maps = _prep_in_maps(query_emb, ctx_embed, W, b)
    res = bass_utils.run_bass_kernel_spmd(nc, in_maps, list(range(NCORES)))
    out = np.concatenate(
        [np.asarray(res.results[c]["out"]) for c in range(NCORES)], axis=0)
    return out.astype(np.float32)



# revision 3
# speedup vs baseline: 1.0882x; 1.0882x over previous
"""Trainium2 Bass kernel for nn_AlignedQuesEmb (v3 — stall-free pipeline).

Reference computation (per batch element b):
    q_dense = relu(query @ W.T + bias)        [Q=48, 300]
    c_dense = relu(ctx @ W.T + bias)          [C=2048, 300]
    scores  = c_dense @ q_dense.T             [C, Q]
    align   = softmax(scores, axis=-1)        (over Q)
    out     = align @ query                   [C, 300]

Sharding: data-parallel over batch. B=64 -> 8 NeuronCores x 8 batches each.

v3 design (vs v2 at ~172us measured steady-state):
  * PE runs at 2.0 GHz sustained (P0 power state, confirmed from MM
    durations (398+N)/2.0). Per-batch PE floor: mm1 36x512 + mm2 12x512
    + mm3 16x304 cols = 29440 cols ~= 14.7us -> ~118us/pass.
  * ctx loads prefetch 2 batches ahead into a fixed 8-slot rotation and
    are issued 2 blocks early, so the first loads of iteration i+1 are
    triggered during iteration i. This kills the boundary PE idle that
    re-throttled the HAM clock gate every iteration (28.7us/iter at
    1.2GHz in v2 traces).
  * mm3(b-1) tiles are interleaved into mm1(b)'s group stream so the
    PSUM normalize drains (ACT/DVE only -- Pool has no PSUM port) spread
    over the whole block instead of back-pressuring a dense mm3 burst
    (v2 lost ~3.6us/batch to pout WAR stalls at 380ns/tile).
  * PSUM: pcd [128,512]x2 banks, psc [48,512]x2, pout [128,304]x4.
  * Elementwise balance per batch: ACT = 6 relu chunks + 4 exp + 9
    normalize scales ~= 10.0us; DVE = 6 relu chunks + 16 recips + 7
    scales ~= 10.9us; both under the 14.7us PE floor.
  * All large HBM traffic is 16-bit as in v2 (fp16 ctx in, fp16 out,
    bf16 E / query); q_dense computed once in fp32 and centered.
"""

import numpy as np
import ml_dtypes

try:
    import concourse.bass as bass  # noqa: F401
except ImportError:
    import sys
    sys.path.insert(0, "/opt/trn_rl_repo")

import concourse.bass as bass
import concourse.tile as tile
from concourse import bacc, mybir
from concourse import bass_utils

F32 = mybir.dt.float32
F16 = mybir.dt.float16
BF16 = mybir.dt.bfloat16
AF = mybir.ActivationFunctionType
AX = mybir.AxisListType
ALU = mybir.AluOpType

B, Q, C, D = 64, 48, 2048, 300
NCORES = 8
BPC = B // NCORES              # batches per core
SHIFT = 60.0                   # constant softmax shift (see module docstring)
KB = 100                       # contraction-band width (3 bands of 100 = D)
EBANDS = [(0, 128), (128, 128), (256, 44)]  # output-dim bands of c_dense
DP = 304                       # query padded with a ones column (row-sum fold)
NT = C // 128                  # c tiles of 128 for mm3


def _build(reps: int = 1, loop_reps: int = 1):
    nc = bacc.Bacc("TRN2", target_bir_lowering=False, debug=False)

    ctxp_d = nc.dram_tensor("ctxp", [BPC, KB, 3 * C], F16, kind="ExternalInput").ap()
    qtf_d = nc.dram_tensor("qtf", [KB, 3 * BPC * Q], F32, kind="ExternalInput").ap()
    wtf_d = nc.dram_tensor("wtf", [KB, 3 * D], F32, kind="ExternalInput").ap()
    wtq_d = nc.dram_tensor("wtq", [KB, 3 * D], F16, kind="ExternalInput").ap()
    qryb_d = nc.dram_tensor("qryb", [Q, BPC * DP], BF16, kind="ExternalInput").ap()
    bias_d = nc.dram_tensor("bias", [D, 1], F32, kind="ExternalInput").ap()
    out_d = nc.dram_tensor("out", [BPC, C, D], F16, kind="ExternalOutput").ap()

    with tile.TileContext(nc) as tc:
        with (
            tc.tile_pool(name="const", bufs=1) as const,
            tc.tile_pool(name="ctx", bufs=BPC) as ctxp,
            tc.tile_pool(name="cdT", bufs=2) as cdp,
            tc.tile_pool(name="esb", bufs=2) as esbp,
            tc.tile_pool(name="osb", bufs=2) as osbp,
            tc.tile_pool(name="rcp", bufs=2) as rcpp,
            tc.tile_pool(name="pcd", bufs=2, space="PSUM") as pcd,
            tc.tile_pool(name="psc", bufs=2, space="PSUM") as psc,
            tc.tile_pool(name="pout", bufs=4, space="PSUM") as pout,
        ):
            # ---- constants ----
            wtq = const.tile([KB, 3 * D], F16, tag="wtq")
            nc.sync.dma_start(wtq[:], wtq_d)
            wtf = const.tile([KB, 3 * D], F32, tag="wtf")
            nc.sync.dma_start(wtf[:], wtf_d)
            qtf = const.tile([KB, 3 * BPC * Q], F32, tag="qtf")
            nc.sync.dma_start(qtf[:], qtf_d)
            qryb = const.tile([Q, BPC * DP], BF16, tag="qryb")
            nc.sync.dma_start(qryb[:], qryb_d)
            bt = []
            for m, (e0, ep) in enumerate(EBANDS):
                btm = const.tile([ep, 1], F32, tag=f"bt{m}")
                nc.sync.dma_start(btm[:], bias_d[e0:e0 + ep, :])
                bt.append(btm)
            negshift = const.tile([Q, 1], F32, tag="negshift")
            nc.vector.memset(negshift[:], -SHIFT)

            # ---- q_denseT for all local batches (exact fp32), centered,
            #      rounded once to fp16 for mm2 ----
            qdT = []
            for m, (e0, ep) in enumerate(EBANDS):
                qf = const.tile([ep, BPC * Q], F32, tag=f"qdTf{m}")
                ps = pcd.tile([128, 512], F32, tag="pcd", name=f"pq_{m}")
                for k in range(3):
                    nc.tensor.matmul(
                        ps[0:ep, 0:BPC * Q],
                        wtf[:, k * D + e0:k * D + e0 + ep],
                        qtf[:, k * BPC * Q:(k + 1) * BPC * Q],
                        start=(k == 0), stop=(k == 2),
                    )
                nc.scalar.activation(qf[:], ps[0:ep, 0:BPC * Q], AF.Relu,
                                     bias=bt[m][:])
                mean = const.tile([ep, 1], F32, tag=f"qmean{m}")
                nc.vector.reduce_sum(mean[:], qf[:], axis=AX.X)
                nc.vector.tensor_scalar_mul(mean[:], mean[:], 1.0 / (BPC * Q))
                nc.vector.tensor_scalar_sub(qf[:], qf[:], mean[:])
                q16 = const.tile([ep, BPC * Q], F16, tag=f"qdT{m}")
                nc.vector.tensor_copy(q16[:], qf[:])
                qdT.append(q16)

            # ctx slots: fixed 8-slot rotation so loads can be issued 2
            # blocks ahead of consumption (incl. across For_i iterations).
            CX = [ctxp.tile([KB, 3 * C], F16, tag="ctx", name=f"ctx_{b}")
                  for b in range(BPC)]

            def load_ctx(slot):
                nc.sync.dma_start(CX[slot][:], ctxp_d[slot])

            # E tiles pre-allocated so the For_i body can read the previous
            # iteration's E[7] at block 0 (loop-carried software pipeline).
            E_tiles = [esbp.tile([Q, C], BF16, tag="E", name=f"E_{b}")
                       for b in range(BPC)]

            def mm1_groups(bb):
                """Yield the 12 (m, g4) PSUM groups of c_denseT; caller
                interleaves mm3 tiles between groups. Drains alternate
                ACT (even) / DVE (odd)."""
                cx = CX[bb % BPC]
                cdT = [cdp.tile([ep, C], F16, tag=f"cd{m}", name=f"cd{m}_{bb}")
                       for m, (e0, ep) in enumerate(EBANDS)]
                gi = 0
                for m, (e0, ep) in enumerate(EBANDS):
                    for g4 in range(4):
                        ps = pcd.tile([128, 512], F32, tag="pcd",
                                      name=f"pcd_{bb}_{m}_{g4}")
                        for k in range(3):
                            nc.tensor.matmul(
                                ps[0:ep, :],
                                wtq[:, k * D + e0:k * D + e0 + ep],
                                cx[:, k * C + g4 * 512:k * C + (g4 + 1) * 512],
                                start=(k == 0), stop=(k == 2),
                            )
                        dst = cdT[m][:, g4 * 512:(g4 + 1) * 512]
                        if gi % 2 == 0:
                            nc.scalar.activation(dst, ps[0:ep, :], AF.Relu,
                                                 bias=bt[m][:])
                        else:
                            nc.vector.tensor_scalar(
                                dst, ps[0:ep, :], bt[m][:], 0.0,
                                ALU.add, ALU.max,
                            )
                        gi += 1
                        yield cdT

            def mm2(bb, cdT, jj):
                """One jj chunk of scoresT -> E = exp(scoresT - SHIFT)."""
                E = E_tiles[bb % BPC]
                qsl = slice((bb % BPC) * Q, (bb % BPC + 1) * Q)
                ps2 = psc.tile([Q, 512], F32, tag="psc", name=f"psc_{bb}_{jj}")
                for m, (e0, ep) in enumerate(EBANDS):
                    nc.tensor.matmul(
                        ps2[:],
                        qdT[m][:, qsl],
                        cdT[m][:, jj * 512:(jj + 1) * 512],
                        start=(m == 0), stop=(m == 2),
                    )
                nc.scalar.activation(
                    E[:, jj * 512:(jj + 1) * 512], ps2[:], AF.Exp,
                    bias=negshift[:],
                )

            TG = 4   # tiles per store DMA

            def mm3_tile(bb, state, t):
                """One c-tile of out = (E.T @ query_pad), normalized by the
                ones-column row sum. Scales: 9 on ACT, 7 on DVE."""
                E = E_tiles[bb % BPC]
                rc, osb = state
                qsl = slice((bb % BPC) * DP, (bb % BPC + 1) * DP)
                po = pout.tile([128, DP], F32, tag="pout",
                               name=f"pout_{bb}_{t}")
                nc.tensor.matmul(
                    po[:], E[:, t * 128:(t + 1) * 128], qryb[:, qsl],
                    start=True, stop=True,
                )
                nc.vector.reciprocal(rc[:, t:t + 1], po[:, D:D + 1])
                dst = osb[:, t * D:(t + 1) * D]
                if t % 2 == 0 or t == 1:
                    nc.scalar.activation(dst, po[:, 0:D], AF.Copy,
                                         scale=rc[:, t:t + 1])
                else:
                    nc.vector.tensor_scalar_mul(dst, po[:, 0:D],
                                                rc[:, t:t + 1])
                if t % TG == TG - 1:
                    g = t // TG
                    # stores go out on the SWDGE (POOL) queue so the SP
                    # HWDGE stream only carries loads.
                    nc.gpsimd.dma_start(
                        out_d[bb % BPC][g * TG * 128:(g + 1) * TG * 128, :]
                            .rearrange("(t p) d -> p t d", p=128),
                        osb[:, g * TG * D:(g + 1) * TG * D]
                            .rearrange("p (t d) -> p t d", t=TG),
                    )

            def mm3_state(bb):
                rc = rcpp.tile([128, NT], F32, tag="rc", name=f"rc_{bb}")
                osb = osbp.tile([128, NT * D], F16, tag="osb", name=f"osb_{bb}")
                return rc, osb

            def block(gb, prev_bb, next_load_slot):
                """One batch block: mm1(gb) with mm3(prev) tiles interleaved
                between PSUM groups, then mm2(gb) chunks with the mm3 tail."""
                if next_load_slot is not None:
                    load_ctx(next_load_slot)
                st = mm3_state(prev_bb) if prev_bb is not None else None
                # interleave plan: 12 mm1 groups; after group i emit mm3
                # tiles so ~12 of 16 land inside mm1, 4 inside mm2.
                emitted = 0
                cdT = None
                gen = mm1_groups(gb)
                for gi in range(12):
                    cdT = next(gen)
                    if st is not None:
                        want = min(12, gi + 1)
                        while emitted < want:
                            mm3_tile(prev_bb, st, emitted)
                            emitted += 1
                for jj in range(4):
                    mm2(gb, cdT, jj)
                    if st is not None and emitted < NT:
                        mm3_tile(prev_bb, st, emitted)
                        emitted += 1

            def one_pass(base):
                prev = None
                for gb in range(BPC):
                    nls = gb + 2 if gb + 2 < BPC else None
                    block(base + gb, prev, nls)
                    prev = base + gb
                # drain the last batch's mm3
                st = mm3_state(prev)
                for t in range(NT):
                    mm3_tile(prev, st, t)

            def one_pass_wrapped():
                """For_i body: batch 7's mm3 wraps to block 0 of the NEXT
                iteration, so the PE never runs an exposed mm3 tail (the
                first iteration emits one garbage out[7] store, overwritten
                by every later iteration)."""
                for gb in range(BPC):
                    block(gb, (gb - 1) % BPC, (gb + 2) % BPC)

            if loop_reps > 1:
                # preload the first two ctx slots before entering the loop
                load_ctx(0)
                load_ctx(1)
                ET = mybir.EngineType
                with tc.For_i(0, loop_reps, 1, staggered_reset=True,
                              hint_engines=(ET.PE, ET.DVE, ET.Activation,
                                            ET.SP, ET.Pool)):
                    one_pass_wrapped()
            else:
                load_ctx(0)
                load_ctx(1)
                for rep in range(reps):
                    one_pass(rep * BPC)
    nc.compile()
    return nc


def _prep_in_maps(query_emb, ctx_embed, W, b):
    query_emb = np.ascontiguousarray(query_emb, dtype=np.float32)
    ctx_embed = np.asarray(ctx_embed, dtype=np.float32)
    W = np.asarray(W, dtype=np.float32)
    wT = np.ascontiguousarray(W.T)                                # [d, e]
    wtf = np.ascontiguousarray(
        wT.reshape(3, KB, D).transpose(1, 0, 2).reshape(KB, 3 * D))
    wtq = wtf.astype(np.float16)
    bias = np.ascontiguousarray(np.asarray(b, np.float32).reshape(D, 1))
    in_maps = []
    for cix in range(NCORES):
        qc = query_emb[cix * BPC:(cix + 1) * BPC]                 # [BPC, Q, D]
        cc = ctx_embed[cix * BPC:(cix + 1) * BPC]                 # [BPC, C, D]
        ctxp = (cc.transpose(0, 2, 1)                             # [BPC, D, C]
                .reshape(BPC, 3, KB, C).transpose(0, 2, 1, 3)
                .reshape(BPC, KB, 3 * C).astype(np.float16))
        qT = qc.transpose(2, 0, 1).reshape(D, BPC * Q)            # [D, BPC*Q]
        qtf = np.ascontiguousarray(
            qT.reshape(3, KB, BPC * Q).transpose(1, 0, 2)
            .reshape(KB, 3 * BPC * Q))
        qp = np.zeros((BPC, Q, DP), np.float32)
        qp[:, :, :D] = qc
        qp[:, :, D] = 1.0     # ones column: mm3 also produces the row sums
        qryb = np.ascontiguousarray(
            qp.transpose(1, 0, 2).reshape(Q, BPC * DP)).astype(ml_dtypes.bfloat16)
        in_maps.append({
            "ctxp": np.ascontiguousarray(ctxp),
            "qtf": qtf,
            "wtf": wtf,
            "wtq": wtq,
            "qryb": qryb,
            "bias": bias,
        })
    return in_maps


_NC_CACHE = {}


def _get_nc(reps: int = 1):
    if reps not in _NC_CACHE:
        _NC_CACHE[reps] = _build(reps)
    return _NC_CACHE[reps]


def kernel(query_emb, ctx_embed, W, b):
    nc = _get_nc()
    in_maps = _prep_in_maps(query_emb, ctx_embed, W, b)
    res = bass_utils.run_bass_kernel_spmd(nc, in_maps, list(range(NCORES)))
    out = np.concatenate(
        [np.asarray(res.results[c]["out"]) for c in range(NCORES)], axis=0)
    return out.astype(np.float32)
